# revision 1
# baseline (speedup 1.0000x reference)
"""Linear attention kernel for 8 Trainium2 NeuronCores.

Sharding: core = 2*b + hg  (b in 0..3 batches, hg in 0..1 head-groups of 8 heads).
Fully data-parallel — no collectives; host sums the two head-group partials per
batch. Each core adds bias/2 so the pair-sum carries the full bias.

Per-core math (T=4096 tokens, CH=512 = 8 heads x 64, DIM=1024):
  qT   = (x @ Wq)^T          c-major [CH, T], elu+1
  k,v  = x @ Wk, x @ Wv      token-major [T, CH], elu+1 on k
  kvT  = v^T k  (per head-pair, diagonal 64-blocks valid), accumulated in PSUM
  z    = ones^T k            [1, CH], accumulated in PSUM
  den  = Zblock^T qT         [8, T]   (Zblock = block-diag expansion of z)
  r    = 1/(den + 1e-6); rbc = E^T r  (broadcast r over each head's 64 rows)
  qsc  = qT * rbc
  M    = kvT^T @ W2  (per 128-row ch-tile; off-diag blocks of kvT zeroed)
  y    = qsc^T @ M + bias/2  token-major [T, DIM]
"""

import sys

sys.path.insert(0, "/opt/trn_rl_repo")

import numpy as np

import concourse.bass as bass
import concourse.mybir as mybir
import concourse.tile as tile
from concourse import bacc

F32 = mybir.dt.float32
BF16 = mybir.dt.bfloat16
AF = mybir.ActivationFunctionType

DIM = 1024      # model dim (contraction for projections)
CH = 512        # per-core channels (8 heads x 64)
P = 128

N_CORES = 8
B, T_FULL = 4, 4096


def build_nc(T=T_FULL):
    NTB = T // 512          # 512-token blocks
    nc = bacc.Bacc(None, target_bir_lowering=False, debug=False)

    xT = nc.declare_dram_parameter("xT", [DIM, T], BF16, isOutput=False)
    w1 = nc.declare_dram_parameter("w1", [DIM, 3 * CH], BF16, isOutput=False)
    w2 = nc.declare_dram_parameter("w2", [CH, DIM], BF16, isOutput=False)
    ec = nc.declare_dram_parameter("ec", [8, CH], BF16, isOutput=False)
    y = nc.declare_dram_parameter("y", [T, DIM], F32, isOutput=True)

    with tile.TileContext(nc) as tc:
        with tc.tile_pool(name="persist", bufs=1) as pp:
            # ---- constants / persistent tiles ----
            ones_col = pp.tile([P, 1], BF16, name="ones_col", tag="ones_col")
            nc.vector.memset(ones_col[:, :], 1.0)

            w1t = []
            for ct in range(8):
                t_ = pp.tile([P, 3 * CH], BF16, name=f"w1_{ct}", tag=f"w1_{ct}")
                nc.sync.dma_start(out=t_[:, :], in_=w1[ct * P:(ct + 1) * P, :])
                w1t.append(t_)

            qt = [
                pp.tile([P, T], BF16, name=f"qt_{j}", tag=f"qt_{j}")
                for j in range(4)
            ]

            kvt = [
                pp.tile([P, P], BF16, name=f"kvt_{j}", tag=f"kvt_{j}")
                for j in range(4)
            ]
            zt = pp.tile([1, CH], BF16, name="zt", tag="zt")

            # prefetch phase-B constants early (off the critical path)
            w2t = []
            for j in range(4):
                t_ = pp.tile([P, DIM], BF16, name=f"w2_{j}", tag=f"w2_{j}")
                nc.sync.dma_start(out=t_[:, :], in_=w2[j * P:(j + 1) * P, :])
                w2t.append(t_)
            ec_sb = pp.tile([8, CH], BF16, name="ec_sb", tag="ec_sb")
            nc.sync.dma_start(out=ec_sb[:, :], in_=ec[:, :])

            phase_a(nc, tc, pp, T, NTB, xT, w1t, qt, kvt, zt, ones_col)
            phase_b(nc, tc, pp, T, NTB, w2t, ec_sb, y, qt, kvt, zt)

    nc.compile()
    return nc


def phase_a(nc, tc, pp, T, NTB, xT, w1t, qt, kvt, zt, ones_col):
    with (
        tc.tile_pool(name="phA_sb", bufs=3) as pa,
        tc.tile_pool(name="xload", bufs=16) as xp,
        tc.tile_pool(name="proj_ps", bufs=6, space="PSUM") as proj_ps,
        tc.tile_pool(name="hold_ps", bufs=1, space="PSUM") as hold_ps,
    ):
            # PSUM accumulators held across all of phase A (one bank each).
            # kvps holds 4 interleaved accumulation regions; zero it up
            # front and accumulate with start=False everywhere (hardware
            # bank-clear on start would wipe sibling regions).
            kvps = hold_ps.tile([P, 4 * P], F32, name="kvps", tag="kvps")
            zps = hold_ps.tile([1, CH], F32, name="zps", tag="zps")
            nc.vector.memset(kvps[:, :], 0.0)

            # ---- phase A: projections + kv/z accumulation ----
            for ib in range(NTB):
                tsl = slice(ib * 512, (ib + 1) * 512)
                xt = []
                for ct in range(8):
                    t_ = xp.tile([P, 512], BF16, name=f"xt_{ib}_{ct}", tag="xt")
                    nc.sync.dma_start(out=t_[:, :], in_=xT[ct * P:(ct + 1) * P, tsl])
                    xt.append(t_)

                # q projection (c-major) with elu+1, into persistent qt
                for j in range(4):
                    qps = proj_ps.tile([P, 512], F32, name=f"qps_{ib}_{j}", tag="proj")
                    for ct in range(8):
                        nc.tensor.matmul(
                            qps[:, :],
                            w1t[ct][:, j * P:(j + 1) * P],
                            xt[ct][:, :],
                            start=(ct == 0),
                            stop=(ct == 7),
                        )
                    m_ = pa.tile([P, 512], F32, name=f"qm_{ib}_{j}", tag="elu_m")
                    e_ = pa.tile([P, 512], F32, name=f"qe_{ib}_{j}", tag="elu_e")
                    r_ = pa.tile([P, 512], F32, name=f"qr_{ib}_{j}", tag="elu_r")
                    nc.vector.tensor_scalar_min(m_[:, :], qps[:, :], 0.0)
                    nc.scalar.activation(e_[:, :], m_[:, :], AF.Exp)
                    nc.scalar.activation(r_[:, :], qps[:, :], AF.Relu)
                    nc.vector.tensor_add(qt[j][:, tsl], e_[:, :], r_[:, :])

                # k, v projections (token-major) per 128-token block
                for t in range(4):
                    tok = slice(t * P, (t + 1) * P)
                    kps = proj_ps.tile([P, 512], F32, name=f"kps_{ib}_{t}", tag="proj")
                    for ct in range(8):
                        nc.tensor.matmul(
                            kps[:, :],
                            xt[ct][:, tok],
                            w1t[ct][:, CH:2 * CH],
                            start=(ct == 0),
                            stop=(ct == 7),
                        )
                    km = pa.tile([P, 512], F32, name=f"km_{ib}_{t}", tag="elu_m")
                    ke = pa.tile([P, 512], F32, name=f"ke_{ib}_{t}", tag="elu_e")
                    kr = pa.tile([P, 512], F32, name=f"kr_{ib}_{t}", tag="elu_r")
                    k_sb = pa.tile([P, 512], BF16, name=f"k_{ib}_{t}", tag="k_sb")
                    nc.vector.tensor_scalar_min(km[:, :], kps[:, :], 0.0)
                    nc.scalar.activation(ke[:, :], km[:, :], AF.Exp)
                    nc.scalar.activation(kr[:, :], kps[:, :], AF.Relu)
                    nc.vector.tensor_add(k_sb[:, :], ke[:, :], kr[:, :])

                    vps = proj_ps.tile([P, 512], F32, name=f"vps_{ib}_{t}", tag="proj")
                    for ct in range(8):
                        nc.tensor.matmul(
                            vps[:, :],
                            xt[ct][:, tok],
                            w1t[ct][:, 2 * CH:3 * CH],
                            start=(ct == 0),
                            stop=(ct == 7),
                        )
                    v_sb = pa.tile([P, 512], BF16, name=f"v_{ib}_{t}", tag="v_sb")
                    nc.scalar.copy(v_sb[:, :], vps[:, :])

                    first = (ib == 0 and t == 0)
                    last = (ib == NTB - 1 and t == 3)
                    # z += ones^T k   [1, 512]
                    nc.tensor.matmul(
                        zps[0:1, :], ones_col[:, :], k_sb[:, :],
                        start=first, stop=last, skip_group_check=True,
                    )
                    # kvT[j] += v_pair^T k_pair   [128, 128] per head-pair.
                    # One accumulation group for the whole packed bank:
                    # start clears the bank once, per-element has_written
                    # handles first-write-overwrite for the other pairs.
                    for j in range(4):
                        csl = slice(j * P, (j + 1) * P)
                        nc.tensor.matmul(
                            kvps[:, csl], v_sb[:, csl], k_sb[:, csl],
                            start=False, stop=(last and j == 3),
                            skip_group_check=True,
                        )

            # ---- evict PSUM accumulators before releasing phase-A pools ----
            for j in range(4):
                nc.vector.memset(kvt[j][:, :], 0.0)
                nc.vector.tensor_copy(
                    kvt[j][0:64, 0:64], kvps[0:64, j * P:j * P + 64]
                )
                nc.vector.tensor_copy(
                    kvt[j][64:128, 64:128],
                    kvps[64:128, j * P + 64:(j + 1) * P],
                )
            nc.vector.tensor_copy(zt[0:1, :], zps[0:1, :])


def phase_b(nc, tc, pp, T, NTB, w2t, ec_sb, y, qt, kvt, zt):
            # ---- phase B setup: Zblock, E, Mstack ----
            Zb, Es = [], []
            for j in range(4):
                zb = pp.tile([P, 8], BF16, name=f"Zb_{j}", tag=f"Zb_{j}")
                nc.vector.memset(zb[:, :], 0.0)
                nc.sync.dma_start(
                    out=zb[0:64, 2 * j:2 * j + 1],
                    in_=zt[0:1, j * P:j * P + 64],
                )
                nc.sync.dma_start(
                    out=zb[64:128, 2 * j + 1:2 * j + 2],
                    in_=zt[0:1, j * P + 64:(j + 1) * P],
                )
                Zb.append(zb)
            for j in range(4):
                Es.append(ec_sb[:, j * P:(j + 1) * P])

            with (
                tc.tile_pool(name="phB_sb", bufs=2) as pb,
                tc.tile_pool(name="qsc_pool", bufs=8) as qp,
                tc.tile_pool(name="phB_ps", bufs=2, space="PSUM") as bps,
                tc.tile_pool(name="y_ps", bufs=3, space="PSUM") as yps_pool,
            ):
                Ms = []
                for j in range(4):
                    ms = pp.tile([P, DIM], BF16, name=f"Ms_{j}", tag=f"Ms_{j}")
                    for h in range(2):
                        hsl = slice(h * 512, (h + 1) * 512)
                        mps = bps.tile([P, 512], F32, name=f"mps_{j}_{h}", tag="m", bufs=1)
                        nc.tensor.matmul(
                            mps[:, :], kvt[j][:, :], w2t[j][:, hsl],
                            start=True, stop=True,
                        )
                        nc.vector.tensor_copy(ms[:, hsl], mps[:, :])
                    Ms.append(ms)

                # ---- phase B main, 3 passes to keep the in-order PE fed ----
                rTs = []
                for ib in range(NTB):
                    tsl = slice(ib * 512, (ib + 1) * 512)
                    dps = bps.tile([8, 512], F32, name=f"dps_{ib}", tag="d")
                    for j in range(4):
                        nc.tensor.matmul(
                            dps[:, :], Zb[j][:, :], qt[j][:, tsl],
                            start=(j == 0), stop=(j == 3),
                        )
                    rf = pb.tile([8, 512], F32, name=f"rf_{ib}", tag="rf")
                    nc.vector.tensor_scalar_add(rf[:, :], dps[:, :], 1e-6)
                    rT = pb.tile([8, 512], BF16, name=f"rT_{ib}", tag="rT", bufs=8)
                    with nc.allow_low_precision(reason="r is O(1e-5); bf16 matches op dtype"):
                        nc.vector.reciprocal(rT[:, :], rf[:, :])
                    rTs.append(rT)

                qscs = []
                for ib in range(NTB):
                    tsl = slice(ib * 512, (ib + 1) * 512)
                    qsc = []
                    for j in range(4):
                        bcp = bps.tile([P, 512], F32, name=f"bcp_{ib}_{j}", tag="bc")
                        nc.tensor.matmul(
                            bcp[:, :], Es[j][:, :], rTs[ib][:, :],
                            start=True, stop=True,
                        )
                        qs = qp.tile(
                            [P, 512], BF16, name=f"qsc_{ib}_{j}", tag="qsc", bufs=32
                        )
                        nc.vector.tensor_mul(qs[:, :], qt[j][:, tsl], bcp[:, :])
                        qsc.append(qs)
                    qscs.append(qsc)

                for ib in range(NTB):
                    qsc = qscs[ib]
                    for t in range(4):
                        tok = slice(t * P, (t + 1) * P)
                        row = (ib * 4 + t) * P
                        y_sb = pb.tile(
                            [P, DIM], F32, name=f"y_{ib}_{t}", tag="y_sb", bufs=3
                        )
                        for h in range(2):
                            hsl = slice(h * 512, (h + 1) * 512)
                            yp = yps_pool.tile(
                                [P, 512], F32, name=f"yps_{ib}_{t}_{h}", tag="y"
                            )
                            for j in range(4):
                                nc.tensor.matmul(
                                    yp[:, :], qsc[j][:, tok], Ms[j][:, hsl],
                                    start=(j == 0), stop=(j == 3),
                                )
                            if h == 0:
                                nc.vector.tensor_copy(y_sb[:, hsl], yp[:, :])
                            else:
                                nc.scalar.copy(y_sb[:, hsl], yp[:, :])
                        nc.sync.dma_start(out=y[row:row + P, :], in_=y_sb[:, :])


_NC_CACHE = {}


def _get_nc(T=T_FULL):
    if T not in _NC_CACHE:
        _NC_CACHE[T] = build_nc(T)
    return _NC_CACHE[T]


def make_in_maps(x, W_qkv, W_out, b_out):
    import ml_dtypes

    bf16 = ml_dtypes.bfloat16
    x = np.asarray(x, dtype=np.float32)
    W_qkv = np.asarray(W_qkv, dtype=np.float32).astype(bf16)
    W_out = np.asarray(W_out, dtype=np.float32).astype(bf16)

    xTs = [np.ascontiguousarray(x[b].T.astype(bf16)) for b in range(B)]
    w1s, w2s = [], []
    for hg in range(2):
        cs = slice(hg * CH, (hg + 1) * CH)
        w1s.append(
            np.ascontiguousarray(
                np.concatenate(
                    [W_qkv[:, cs],
                     W_qkv[:, DIM + hg * CH:DIM + (hg + 1) * CH],
                     W_qkv[:, 2 * DIM + hg * CH:2 * DIM + (hg + 1) * CH]],
                    axis=1,
                )
            )
        )
        w2s.append(np.ascontiguousarray(W_out[cs, :]))
    ecm = make_ec().astype(bf16)

    in_maps = []
    for core in range(N_CORES):
        b, hg = core // 2, core % 2
        in_maps.append({"xT": xTs[b], "w1": w1s[hg], "w2": w2s[hg], "ec": ecm})
    return in_maps


def make_ec():
    """E selector: ec[h, j*128+p] = 1 iff head-of-partition-p-in-tile-j == h."""
    ecm = np.zeros((8, CH), dtype=np.float32)
    for j in range(4):
        ecm[2 * j, j * P:j * P + 64] = 1.0
        ecm[2 * j + 1, j * P + 64:(j + 1) * P] = 1.0
    return ecm


def kernel(x, W_qkv, W_out, b_out):
    from concourse.bass_utils import run_bass_kernel_spmd

    nc = _get_nc(T_FULL)
    in_maps = make_in_maps(x, W_qkv, W_out, b_out)
    res = run_bass_kernel_spmd(nc, in_maps, core_ids=list(range(N_CORES))).results
    bo = np.asarray(b_out, dtype=np.float32)
    out = np.empty((B, T_FULL, DIM), dtype=np.float32)
    for b in range(B):
        out[b] = res[2 * b]["y"] + res[2 * b + 1]["y"] + bo
    return out



# revision 4
# speedup vs baseline: 1.1286x; 1.1286x over previous
"""Linear attention kernel for 8 Trainium2 NeuronCores.

Sharding: core = 2*b + hg  (b in 0..3 batches, hg in 0..1 head-groups of 8 heads).
Fully data-parallel — no collectives; host sums the two head-group partials per
batch (f32) and adds the bias.

Per-core math (T=4096 tokens, CH=512 = 8 heads x 64, DIM=1024):
  Phase 1 (per 512-token block): k,v = x @ Wk, x @ Wv token-major; elu+1 on k;
    qT = (x @ Wq)^T c-major with elu+1 (persisted for phase 2);
    kvT += v^T k per head-pair (diagonal 64-blocks), z += ones^T k, in PSUM.
  Boundary: evict kvT/z; M = kvT^T @ W2; Zb = block-diag expansion of z.
  Phase 2 (per 512-token block): den = Zb^T qT [8,T-blk]; r = 1/(den+1e-6)
    computed in a [128,32] partition-blocked layout (DMA reshape both ways);
    rbc = E^T r (broadcast over each head's 64 rows); qsc = qT * rbc;
    y = qsc^T @ M, written bf16 (host accumulates in f32).
"""

import sys

sys.path.insert(0, "/opt/trn_rl_repo")

import numpy as np

import concourse.bass as bass
import concourse.mybir as mybir
import concourse.tile as tile
from concourse import bacc

F32 = mybir.dt.float32
BF16 = mybir.dt.bfloat16
AF = mybir.ActivationFunctionType

DIM = 1024      # model dim (contraction for projections)
CH = 512        # per-core channels (8 heads x 64)
P = 128

N_CORES = 8
B, T_FULL = 4, 4096


def build_nc(T=T_FULL):
    NTB = T // 512          # 512-token blocks
    nc = bacc.Bacc(None, target_bir_lowering=False, debug=False)

    xT = nc.declare_dram_parameter("xT", [DIM, T], BF16, isOutput=False)
    w1 = nc.declare_dram_parameter("w1", [DIM, 3 * CH], BF16, isOutput=False)
    w2 = nc.declare_dram_parameter("w2", [CH, DIM], BF16, isOutput=False)
    ec = nc.declare_dram_parameter("ec", [8, CH], BF16, isOutput=False)
    y = nc.declare_dram_parameter("y", [T, DIM], BF16, isOutput=True)

    with tile.TileContext(nc) as tc:
        with tc.tile_pool(name="persist", bufs=1) as pp:
            ones_col = pp.tile([P, 1], BF16, name="ones_col", tag="ones_col")
            nc.vector.memset(ones_col[:, :], 1.0)

            # w1 split: kv columns first (phase-1 critical path), q columns
            # arrive during the first k/v matmul groups.
            w1kv, w1q = [], []
            for ct in range(8):
                t_ = pp.tile([P, 2 * CH], BF16, name=f"w1kv_{ct}", tag=f"w1kv_{ct}")
                nc.sync.dma_start(out=t_[:, :], in_=w1[ct * P:(ct + 1) * P, CH:3 * CH])
                w1kv.append(t_)
            for ct in range(8):
                t_ = pp.tile([P, CH], BF16, name=f"w1q_{ct}", tag=f"w1q_{ct}")
                nc.sync.dma_start(out=t_[:, :], in_=w1[ct * P:(ct + 1) * P, 0:CH])
                w1q.append(t_)

            # persistent q (c-major, bf16) for all blocks: 32 x [128, 512]
            qt = [
                [
                    pp.tile([P, 512], BF16, name=f"qt_{ib}_{j}", tag=f"qt_{ib}_{j}")
                    for j in range(4)
                ]
                for ib in range(NTB)
            ]

            kvt = [
                pp.tile([P, P], BF16, name=f"kvt_{j}", tag=f"kvt_{j}")
                for j in range(4)
            ]
            zt = pp.tile([1, CH], BF16, name="zt", tag="zt")

            w2t, Ms = [], []
            for j in range(4):
                w2t.append(pp.tile([P, DIM], BF16, name=f"w2_{j}", tag=f"w2_{j}"))
                Ms.append(pp.tile([P, DIM], BF16, name=f"Ms_{j}", tag=f"Ms_{j}"))
            ec_sb = pp.tile([8, CH], BF16, name="ec_sb", tag="ec_sb")

            Zb = [
                pp.tile([P, 8], BF16, name=f"Zb_{j}", tag=f"Zb_{j}")
                for j in range(4)
            ]

            phase1(nc, tc, pp, T, NTB, xT, w2, ec, w1kv, w1q, qt, kvt, zt,
                   w2t, ec_sb, ones_col)

            # ---- boundary: Zb from zt; M = kvt^T @ W2 ----
            for j in range(4):
                nc.vector.memset(Zb[j][:, :], 0.0)
                nc.sync.dma_start(
                    out=Zb[j][0:64, 2 * j:2 * j + 1],
                    in_=zt[0:1, j * P:j * P + 64],
                )
                nc.sync.dma_start(
                    out=Zb[j][64:128, 2 * j + 1:2 * j + 2],
                    in_=zt[0:1, j * P + 64:(j + 1) * P],
                )

            phase2(nc, tc, pp, T, NTB, y, qt, kvt, zt, w2t, ec_sb, Ms, Zb)

    nc.compile()
    return nc


def phase1(nc, tc, pp, T, NTB, xT, w2, ec, w1kv, w1q, qt, kvt, zt,
           w2t, ec_sb, ones_col):
    with (
        tc.tile_pool(name="ph1_sb", bufs=3) as pa,
        tc.tile_pool(name="kv_sb", bufs=3) as kvp,
        tc.tile_pool(name="xload", bufs=24) as xp,
        tc.tile_pool(name="proj_ps", bufs=6, space="PSUM") as proj_ps,
        tc.tile_pool(name="hold_ps", bufs=1, space="PSUM") as hold_ps,
    ):
        # PSUM accumulators held across all of phase 1 (one bank each).
        kvps = hold_ps.tile([P, 4 * P], F32, name="kvps", tag="kvps")
        zps = hold_ps.tile([1, CH], F32, name="zps", tag="zps")
        nc.vector.memset(kvps[:, :], 0.0)

        pending = []            # deferred z/kv matmuls (closures)

        def flush_pending():
            while pending:
                pending.pop(0)()

        for ib in range(NTB):
            xt = []
            for ct in range(8):
                t_ = xp.tile([P, 512], BF16, name=f"xt_{ib}_{ct}", tag="xt")
                nc.sync.dma_start(
                    out=t_[:, :], in_=xT[ct * P:(ct + 1) * P, ib * 512:(ib + 1) * 512]
                )
                xt.append(t_)

            # k/v projections (token-major) per 128-token chunk, with the
            # z/kv matmuls of the previous chunk interleaved after each
            # vps group so they never wait on the elu chain.
            for t in range(4):
                tok = slice(t * P, (t + 1) * P)
                kps = proj_ps.tile([P, 512], F32, name=f"kps_{ib}_{t}", tag="proj")
                for ct in range(8):
                    nc.tensor.matmul(
                        kps[:, :], xt[ct][:, tok], w1kv[ct][:, 0:CH],
                        start=(ct == 0), stop=(ct == 7),
                    )
                km = pa.tile([P, 512], BF16, name=f"km_{ib}_{t}", tag="elu_m")
                ke = pa.tile([P, 512], BF16, name=f"ke_{ib}_{t}", tag="elu_e")
                kr = pa.tile([P, 512], BF16, name=f"kr_{ib}_{t}", tag="elu_r")
                k_sb = kvp.tile([P, 512], BF16, name=f"k_{ib}_{t}", tag="k_sb")
                nc.vector.tensor_scalar_min(km[:, :], kps[:, :], 0.0)
                nc.scalar.activation(ke[:, :], km[:, :], AF.Exp)
                nc.scalar.activation(kr[:, :], kps[:, :], AF.Relu)
                nc.vector.tensor_add(k_sb[:, :], ke[:, :], kr[:, :])

                vps = proj_ps.tile([P, 512], F32, name=f"vps_{ib}_{t}", tag="proj")
                for ct in range(8):
                    nc.tensor.matmul(
                        vps[:, :], xt[ct][:, tok], w1kv[ct][:, CH:2 * CH],
                        start=(ct == 0), stop=(ct == 7),
                    )
                v_sb = kvp.tile([P, 512], BF16, name=f"v_{ib}_{t}", tag="v_sb")
                nc.vector.tensor_copy(v_sb[:, :], vps[:, :])

                flush_pending()

                def defer(ib=ib, t=t, k_sb=k_sb, v_sb=v_sb):
                    first = (ib == 0 and t == 0)
                    last = (ib == NTB - 1 and t == 3)
                    # z += ones^T k   [1, 512]
                    nc.tensor.matmul(
                        zps[0:1, :], ones_col[:, :], k_sb[:, :],
                        start=first, stop=last, skip_group_check=True,
                    )
                    # kvT[j] += v_pair^T k_pair  [128,128] per head-pair; one
                    # accumulation region per j inside the pre-zeroed bank.
                    for j in range(4):
                        csl = slice(j * P, (j + 1) * P)
                        nc.tensor.matmul(
                            kvps[:, csl], v_sb[:, csl], k_sb[:, csl],
                            start=False, stop=(last and j == 3),
                            skip_group_check=True,
                        )
                pending.append(defer)

            # q projection (c-major) with elu+1, into persistent qt
            for j in range(4):
                qps = proj_ps.tile([P, 512], F32, name=f"qps_{ib}_{j}", tag="proj")
                for ct in range(8):
                    nc.tensor.matmul(
                        qps[:, :],
                        w1q[ct][:, j * P:(j + 1) * P],
                        xt[ct][:, :],
                        start=(ct == 0), stop=(ct == 7),
                    )
                if j == 0:
                    flush_pending()
                qm = pa.tile([P, 512], BF16, name=f"qm_{ib}_{j}", tag="elu_m")
                qe = pa.tile([P, 512], BF16, name=f"qe_{ib}_{j}", tag="elu_e")
                qr = pa.tile([P, 512], BF16, name=f"qr_{ib}_{j}", tag="elu_r")
                nc.vector.tensor_scalar_min(qm[:, :], qps[:, :], 0.0)
                nc.scalar.activation(qe[:, :], qm[:, :], AF.Exp)
                nc.scalar.activation(qr[:, :], qps[:, :], AF.Relu)
                nc.vector.tensor_add(qt[ib][j][:, :], qe[:, :], qr[:, :])

            if ib == 0:
                # stage phase-2 constants off the critical path
                for j in range(4):
                    nc.sync.dma_start(
                        out=w2t[j][:, :], in_=w2[j * P:(j + 1) * P, :]
                    )
                nc.sync.dma_start(out=ec_sb[:, :], in_=ec[:, :])

        flush_pending()

        # ---- evict PSUM accumulators before releasing phase-1 pools ----
        for j in range(4):
            nc.vector.memset(kvt[j][:, :], 0.0)
            nc.vector.tensor_copy(
                kvt[j][0:64, 0:64], kvps[0:64, j * P:j * P + 64]
            )
            nc.vector.tensor_copy(
                kvt[j][64:128, 64:128],
                kvps[64:128, j * P + 64:(j + 1) * P],
            )
        nc.vector.tensor_copy(zt[0:1, :], zps[0:1, :])


def phase2(nc, tc, pp, T, NTB, y, qt, kvt, zt, w2t, ec_sb, Ms, Zb):
    Es = [ec_sb[:, j * P:(j + 1) * P] for j in range(4)]

    with tc.tile_pool(name="m_ps", bufs=2, space="PSUM") as mps_pool:
        # M = kvt^T @ W2 (per 128-row ch-tile; off-diag blocks of kvt zeroed)
        for j in range(4):
            for h in range(2):
                hsl = slice(h * 512, (h + 1) * 512)
                mps = mps_pool.tile([P, 512], F32, name=f"mps_{j}_{h}", tag="m")
                nc.tensor.matmul(
                    mps[:, :], kvt[j][:, :], w2t[j][:, hsl],
                    start=True, stop=True,
                )
                nc.vector.tensor_copy(Ms[j][:, hsl], mps[:, :])

    with (
        tc.tile_pool(name="ph2_sb", bufs=2) as pb,
        tc.tile_pool(name="qsc_pool", bufs=8) as qp,
        tc.tile_pool(name="d_ps", bufs=2, space="PSUM") as dps_pool,
        tc.tile_pool(name="y_ps", bufs=3, space="PSUM") as yps_pool,
    ):
        # Pipeline per 512-token block: den -> r (reshaped [128,32]) -> rT
        # -> rbc (bc matmul) -> qsc -> y matmuls.  The consumer stage runs
        # one block behind the den stage so the PE never waits on the
        # den->rT DMA/reciprocal chain.
        rTs = [None] * NTB

        def den_stage(ib):
            dps = dps_pool.tile([8, 512], F32, name=f"dps_{ib}", tag="d")
            for j in range(4):
                nc.tensor.matmul(
                    dps[:, :], Zb[j][:, :], qt[ib][j][:, :],
                    start=(j == 0), stop=(j == 3),
                )
            den_sb = pb.tile([8, 512], F32, name=f"den_{ib}", tag="den_sb")
            nc.vector.tensor_scalar_add(den_sb[:, :], dps[:, :], 1e-6)
            # partition-blocked reshape: [8 heads, 512 tok] ->
            # [128 = head*16 + tok//32, 32 = tok%32]
            den_rs = pb.tile([P, 32], F32, name=f"drs_{ib}", tag="den_rs")
            nc.sync.dma_start(out=den_rs[:, :], in_=den_sb[:, :])
            rr = pb.tile([P, 32], BF16, name=f"rr_{ib}", tag="rr")
            with nc.allow_low_precision(reason="r is O(1e-5); bf16 matches op dtype"):
                nc.vector.reciprocal(rr[:, :], den_rs[:, :])
            rT = pb.tile([8, 512], BF16, name=f"rT_{ib}", tag="rT", bufs=3)
            nc.sync.dma_start(out=rT[:, :], in_=rr[:, :])
            rTs[ib] = rT

        def out_stage(ib):
            qsc = []
            for j in range(4):
                bcp = dps_pool.tile([P, 512], F32, name=f"bcp_{ib}_{j}", tag="bc")
                nc.tensor.matmul(
                    bcp[:, :], Es[j][:, :], rTs[ib][:, :], start=True, stop=True,
                )
                qs = qp.tile([P, 512], BF16, name=f"qsc_{ib}_{j}", tag="qsc")
                nc.vector.tensor_mul(qs[:, :], qt[ib][j][:, :], bcp[:, :])
                qsc.append(qs)

            for t in range(4):
                tok = slice(t * P, (t + 1) * P)
                row = (ib * 4 + t) * P
                y_sb = pb.tile([P, DIM], BF16, name=f"y_{ib}_{t}", tag="y_sb",
                               bufs=3)
                for h in range(2):
                    hsl = slice(h * 512, (h + 1) * 512)
                    yp = yps_pool.tile([P, 512], F32, name=f"yps_{ib}_{t}_{h}",
                                       tag="y")
                    for j in range(4):
                        nc.tensor.matmul(
                            yp[:, :], qsc[j][:, tok], Ms[j][:, hsl],
                            start=(j == 0), stop=(j == 3),
                        )
                    if h == 0:
                        nc.vector.tensor_copy(y_sb[:, hsl], yp[:, :])
                    else:
                        nc.scalar.copy(y_sb[:, hsl], yp[:, :])
                nc.sync.dma_start(out=y[row:row + P, :], in_=y_sb[:, :])

        for ib in range(NTB):
            den_stage(ib)
            if ib >= 1:
                out_stage(ib - 1)
        out_stage(NTB - 1)


_NC_CACHE = {}


def _get_nc(T=T_FULL):
    if T not in _NC_CACHE:
        _NC_CACHE[T] = build_nc(T)
    return _NC_CACHE[T]


def make_in_maps(x, W_qkv, W_out, b_out):
    import ml_dtypes

    bf16 = ml_dtypes.bfloat16
    x = np.asarray(x, dtype=np.float32)
    W_qkv = np.asarray(W_qkv, dtype=np.float32).astype(bf16)
    W_out = np.asarray(W_out, dtype=np.float32).astype(bf16)

    xTs = [np.ascontiguousarray(x[b].T.astype(bf16)) for b in range(B)]
    w1s, w2s = [], []
    for hg in range(2):
        cs = slice(hg * CH, (hg + 1) * CH)
        w1s.append(
            np.ascontiguousarray(
                np.concatenate(
                    [W_qkv[:, cs],
                     W_qkv[:, DIM + hg * CH:DIM + (hg + 1) * CH],
                     W_qkv[:, 2 * DIM + hg * CH:2 * DIM + (hg + 1) * CH]],
                    axis=1,
                )
            )
        )
        w2s.append(np.ascontiguousarray(W_out[cs, :]))
    ecm = make_ec().astype(bf16)

    in_maps = []
    for core in range(N_CORES):
        b, hg = core // 2, core % 2
        in_maps.append({"xT": xTs[b], "w1": w1s[hg], "w2": w2s[hg], "ec": ecm})
    return in_maps


def make_ec():
    """E selector: ec[h, j*128+p] = 1 iff head-of-partition-p-in-tile-j == h."""
    ecm = np.zeros((8, CH), dtype=np.float32)
    for j in range(4):
        ecm[2 * j, j * P:j * P + 64] = 1.0
        ecm[2 * j + 1, j * P + 64:(j + 1) * P] = 1.0
    return ecm


def kernel(x, W_qkv, W_out, b_out):
    from concourse.bass_utils import run_bass_kernel_spmd

    nc = _get_nc(T_FULL)
    in_maps = make_in_maps(x, W_qkv, W_out, b_out)
    res = run_bass_kernel_spmd(nc, in_maps, core_ids=list(range(N_CORES))).results
    bo = np.asarray(b_out, dtype=np.float32)
    out = np.empty((B, T_FULL, DIM), dtype=np.float32)
    for b in range(B):
        out[b] = (res[2 * b]["y"].astype(np.float32)
                  + res[2 * b + 1]["y"].astype(np.float32) + bo)
    return out
